# revision 7
# baseline (speedup 1.0000x reference)
"""Trainium2 Bass kernel: causal multi-head self-attention with QKV/out projections.

Reference computation (per (batch b, column c) independently):
    q = X_q @ Wq.T + bq ; k = X_k @ Wk.T + bk ; v = X_v @ Wv.T + bv
    per head h (D=64): S = q_h @ k_h.T / sqrt(D);  causal softmax;  O_h = P @ v_h
    out = concat_h(O_h) @ Wo.T + bo

Sharding: data-parallel over the B*C = 32 independent (b, c) attention
problems -> 4 per NeuronCore across 8 cores. Weights are broadcast.

Per-core kernel layout ("transposed space"):
  - inputs arrive pre-transposed from the host as X^T [E, S] per (b, c); all
    matmul operands are fp16 (fp32 PSUM accumulation)
  - projections compute Q^T, K^T ([j, t]) and V natural [t, j]
  - scores are computed transposed: S^T[k, q] = sum_d K^T[d,k] Q^T[d,q].
    Adjacent head-pair matmuls use PE row groups 0-63 / 64-127; both heads
    share one 2-bank PSUM tile so exp covers the pair in one instruction.
    Blocks above the causal diagonal are skipped, diagonal blocks trimmed.
  - P^T = exp(0.125 * S^T) on the scalar engine (max-subtraction skipped:
    |0.125*s| <= ~6 here, exact as the reference after its max-subtraction);
    the diagonal triangle is zeroed by a gpsimd affine_select (idle engine)
  - AV with "wide ones": the V stationary per head is [128, 64 V-cols |
    64 ones-cols], so the AV PSUM [128, 512] holds O^T on rows 0-63 and the
    softmax denominator l replicated on rows 64-127. The reciprocal runs
    directly on PSUM rows 64-127 into a 64-partition SBUF tile and one
    tensor_mul writes O^T/l -- no PSUM->SBUF staging copy and no PE
    broadcast matmul.
  - out projection is computed TRANSPOSED (out^T[e, t] = sum_d Wo^T[d,e]^T..)
    with Wo^T chunks stationary and O^T streamed, so the output bias
    (bo + Wo@bv, folded on host) is a per-partition scalar folded into the
    PSUM->SBUF copy on the DVE; the host transposes the [E, S] result back
  - emission interleaves fine-grained "filler" PE work (next-(b,c)
    projections, current-(b,c) out-projection) one matmul at a time into the
    attention ki loop, so the in-order PE stream has independent work during
    exp-paced gaps and unit-boundary normalize stalls
Built on Bacc (walrus here allows 1 sync-wait per instruction; Bacc's
generate_event_semaphores splits them).
"""

import threading
from collections import deque

import numpy as np

B, C, S, E, H = 4, 8, 1024, 512, 8
D = E // H            # 64
NCORES = 8
BC = (B * C) // NCORES  # 4 (b,c) pairs per core
NEC = E // 128        # 4 e-chunks
NTT = S // 128        # 8 token tiles of 128
NKT = S // 128        # 8 key tiles of 128
NQT = S // 512        # 2 query tiles of 512
HPC = 128 // D        # 2 heads per 128-row chunk

_SCALE = 1.0 / 8.0    # 1/sqrt(D)


def build_nc(reps=1):
    import concourse.mybir as mybir
    from concourse.bacc import Bacc
    from concourse.tile import TileContext

    F32 = mybir.dt.float32
    F16 = mybir.dt.float16
    Exp = mybir.ActivationFunctionType.Exp

    nc = Bacc()

    q_in = nc.declare_dram_parameter("q_in", [BC, E, S], F16, isOutput=False)
    k_in = nc.declare_dram_parameter("k_in", [BC, E, S], F16, isOutput=False)
    v_in = nc.declare_dram_parameter("v_in", [BC, E, S], F16, isOutput=False)
    wq_d = nc.declare_dram_parameter("wqT", [E, E], F16, isOutput=False)
    wk_d = nc.declare_dram_parameter("wkT", [E, E], F16, isOutput=False)
    wv_d = nc.declare_dram_parameter("wvT", [E, E], F16, isOutput=False)
    wo_d = nc.declare_dram_parameter("woT", [E, E], F16, isOutput=False)
    bq_d = nc.declare_dram_parameter("bq", [E], F32, isOutput=False)
    bk_d = nc.declare_dram_parameter("bk", [E], F32, isOutput=False)
    bo_d = nc.declare_dram_parameter("bo_eff", [E], F32, isOutput=False)
    out_d = nc.declare_dram_parameter("out", [BC, E, S], F32, isOutput=True)

    with TileContext(nc) as tc:
        with (
            tc.tile_pool(name="const", bufs=1) as constp,
            tc.tile_pool(name="wts", bufs=1) as wtsp,
            tc.tile_pool(name="xt", bufs=1) as xtp,
            tc.tile_pool(name="qkv", bufs=1) as qkvp,
            tc.tile_pool(name="pt", bufs=12) as ptp,
            tc.tile_pool(name="sml", bufs=4) as smlp,
            tc.tile_pool(name="ob", bufs=4) as obp,
            tc.tile_pool(name="psmm", bufs=2, space="PSUM") as psmm,
            tc.tile_pool(name="pssc", bufs=2, space="PSUM") as pssc,
            tc.tile_pool(name="psav", bufs=2, space="PSUM") as psav,
        ):
            # ---------------- one-time constants ----------------
            bq_sb = constp.tile([128, NEC], F32, name="bq_sb")
            nc.sync.dma_start(out=bq_sb, in_=bq_d[:].rearrange("(c p) -> p c", p=128))
            bk_sb = constp.tile([128, NEC], F32, name="bk_sb")
            nc.sync.dma_start(out=bk_sb, in_=bk_d[:].rearrange("(c p) -> p c", p=128))
            bo_sb = constp.tile([128, NEC], F32, name="bo_sb")
            nc.sync.dma_start(out=bo_sb, in_=bo_d[:].rearrange("(c p) -> p c", p=128))

            # -------- weights (host-pre-transposed): wXt[ec][p, j] = W[j, 128ec+p]
            wts = {}

            def load_weight(wname, wd):
                wts[wname] = []
                for ec in range(NEC):
                    wt = wtsp.tile([128, E], F16, name=f"{wname}T{ec}",
                                   tag=f"{wname}T{ec}")
                    nc.sync.dma_start(out=wt, in_=wd[128 * ec:128 * (ec + 1), :])
                    wts[wname].append(wt)

            load_weight("wq", wq_d)
            _late_weights = [("wk", wk_d), ("wv", wv_d), ("wo", wo_d)]

            def make_state(bc):
                st = {"bc": bc}
                st["xt"] = {}
                for iname, ind in (("q", q_in), ("k", k_in), ("v", v_in)):
                    st["xt"][iname] = []
                    for ec in range(NEC):
                        t = xtp.tile([128, S], F16, name=f"xt_{iname}{ec}_{bc}",
                                     tag=f"xt_{iname}{ec}", bufs=2)
                        nc.sync.dma_start(
                            out=t, in_=ind[bc, 128 * ec:128 * (ec + 1), :])
                        st["xt"][iname].append(t)
                st["qT"] = [qkvp.tile([128, S], F16, name=f"qT{jc}_{bc}",
                                      tag=f"qT{jc}", bufs=2) for jc in range(NEC)]
                st["kT"] = [qkvp.tile([128, S], F16, name=f"kT{jc}_{bc}",
                                      tag=f"kT{jc}", bufs=2) for jc in range(NEC)]
                # per head h: cols 128h..128h+63 = V d-cols, 128h+64..127 = ones
                st["vsb"] = [qkvp.tile([128, H * 128], F16,
                                       name=f"vsb{tt}_{bc}", tag=f"vsb{tt}",
                                       bufs=2) for tt in range(NTT)]
                st["oT"] = [qkvp.tile([128, S], F16, name=f"oT{ec}_{bc}",
                                      tag=f"oT{ec}", bufs=2) for ec in range(NEC)]
                return st

            # ---- filler generators: yield after each PE matmul so the
            # driver can interleave them one matmul at a time ----
            def gen_qkproj(st, jc, qt):
                bc = st["bc"]
                for dst, wname, xname, bias in (
                    (st["qT"], "wq", "q", bq_sb), (st["kT"], "wk", "k", bk_sb)
                ):
                    ps = psmm.tile([128, 512], F32,
                                   name=f"prj_{wname}{jc}{qt}_{bc}", tag="mm")
                    for ec in range(NEC):
                        nc.tensor.matmul(
                            ps,
                            lhsT=(wts[wname][ec][:, 128 * jc:128 * (jc + 1)]),
                            rhs=(st["xt"][xname][ec][:, 512 * qt:512 * (qt + 1)]),
                            start=(ec == 0), stop=(ec == NEC - 1),
                        )
                        yield
                    nc.vector.tensor_scalar_add(
                        dst[jc][:, 512 * qt:512 * (qt + 1)], ps,
                        bias[:, jc:jc + 1])

            def gen_vproj(st, tt):
                bc = st["bc"]
                v3 = st["vsb"][tt].rearrange("p (h c) -> p h c", c=128)
                if bc < 2:
                    # ones region: written once per physical buffer (tags
                    # rotate through 2 bufs; V copies never touch it)
                    nc.gpsimd.memset(v3[:, :, D:128], 1.0)
                ps = psmm.tile([128, 512], F32, name=f"prj_v{tt}_{bc}", tag="mm")
                for ec in range(NEC):
                    nc.tensor.matmul(
                        ps,
                        lhsT=(st["xt"]["v"][ec][:, 128 * tt:128 * (tt + 1)]),
                        rhs=wts["wv"][ec],
                        start=(ec == 0), stop=(ec == NEC - 1),
                    )
                    yield
                nc.vector.tensor_copy(
                    v3[:, :, 0:D], ps.rearrange("p (h c) -> p h c", c=D))

            def gen_outproj(st, et, ts):
                # out^T[128et+e', 512ts+t'] = sum_d Wo^T[d, e] O^T[d, t] + bo
                bc = st["bc"]
                ps = psmm.tile([128, 512], F32, name=f"op{et}{ts}_{bc}", tag="mm")
                for ec in range(NEC):
                    nc.tensor.matmul(
                        ps,
                        lhsT=(wts["wo"][ec][:, 128 * et:128 * (et + 1)]),
                        rhs=(st["oT"][ec][:, 512 * ts:512 * (ts + 1)]),
                        start=(ec == 0), stop=(ec == NEC - 1),
                    )
                    yield
                outsb = obp.tile([128, 512], F32, name=f"outsb{et}{ts}_{bc}",
                                 tag="ob")
                nc.vector.tensor_scalar_add(outsb, ps, bo_sb[:, et:et + 1])
                nc.sync.dma_start(
                    out=out_d[bc, 128 * et:128 * (et + 1),
                              512 * ts:512 * (ts + 1)],
                    in_=outsb)

            # ---- filler scheduling ----
            # Only one generator is ever mid-flight (holds a psmm "mm" tag
            # buffer); force_gens finishes the live one first so tag
            # rotation never interleaves two partial accumulations.
            pending = deque()
            live = {"gen": None}

            def step_filler(n=1):
                for _ in range(n):
                    while live["gen"] is None and pending:
                        live["gen"] = pending.popleft()
                    g = live["gen"]
                    if g is None:
                        return
                    try:
                        next(g)
                    except StopIteration:
                        live["gen"] = None

            def run_to_end(g):
                while True:
                    try:
                        next(g)
                    except StopIteration:
                        return

            def force_gens(gens):
                # complete these specific generators NOW (deps of the next
                # pipeline stage); exhausted entries later pop as no-ops
                if live["gen"] is not None:
                    run_to_end(live["gen"])
                    live["gen"] = None
                for g in gens:
                    run_to_end(g)

            def drain_all():
                while live["gen"] is not None or pending:
                    step_filler()

            # ---- attention unit (pr = head pair, qt = 512-query stripe) ----
            def emit_attn_unit(st, pr, qt):
                bc = st["bc"]
                qT, kT, vsb, oT_hat = st["qT"], st["kT"], st["vsb"], st["oT"]
                kmax = NKT - 1 if qt == NQT - 1 else (512 * (qt + 1)) // 128 - 1
                avp = [psav.tile([128, 512], F32,
                                 name=f"av{pr}{qt}{hf}_{bc}", tag="av")
                       for hf in range(HPC)]

                def scores(ki):
                    rr = max(128 * ki - 512 * qt, 0)
                    sps = pssc.tile([128, 2 * 512], F32,
                                    name=f"sc{pr}{qt}{ki}_{bc}", tag="sc")
                    for hf in range(HPC):
                        row0 = D * hf
                        nc.tensor.matmul(
                            sps[:, 512 * hf + rr:512 * (hf + 1)],
                            lhsT=(kT[pr][row0:row0 + D,
                                         128 * ki:128 * (ki + 1)]),
                            rhs=(qT[pr][row0:row0 + D,
                                        512 * qt + rr:512 * (qt + 1)]),
                            start=True, stop=True,
                        )
                    return sps

                def consume(ki, sps):
                    rr = max(128 * ki - 512 * qt, 0)
                    diag = 128 * ki - 512 * qt >= 0
                    pt = ptp.tile([128, 2 * 512], F16,
                                  name=f"pt{pr}{qt}{ki}_{bc}", tag="pt")
                    s3 = sps.rearrange("p (h q) -> p h q", h=HPC)
                    p3 = pt.rearrange("p (h q) -> p h q", h=HPC)
                    nc.scalar.activation(
                        p3[:, :, rr:512], s3[:, :, rr:512], Exp, scale=_SCALE)
                    if diag:
                        # zero the upper triangle of the exp'd diagonal
                        # strips: keep where q' >= k'
                        nc.gpsimd.affine_select(
                            out=p3[:, :, rr:rr + 128],
                            in_=p3[:, :, rr:rr + 128],
                            compare_op=mybir.AluOpType.is_ge,
                            fill=0.0,
                            base=0, pattern=[[0, HPC], [1, 128]],
                            channel_multiplier=-1,
                        )
                    for hf in range(HPC):
                        h = HPC * pr + hf
                        nc.tensor.matmul(
                            avp[hf][:, rr:512],
                            lhsT=(vsb[ki][:, 128 * h:128 * (h + 1)]),
                            rhs=(pt[:, 512 * hf + rr:512 * (hf + 1)]),
                            start=(ki == 0), stop=(ki == kmax),
                        )

                # software pipeline: PE one ki ahead on scores; pull one
                # filler matmul per ki so exp-paced gaps stay filled
                prev = scores(0)
                for ki in range(1, kmax + 1):
                    step_filler(1)
                    cur = scores(ki)
                    consume(ki - 1, prev)
                    prev = cur
                consume(kmax, prev)
                # normalize straight from PSUM: rows 64-127 hold l
                for hf in range(HPC):
                    linv = smlp.tile([64, 512], F16,
                                     name=f"linv{pr}{qt}{hf}_{bc}",
                                     tag=f"linv{hf}")
                    with nc.allow_low_precision(reason="fp16 softmax recip"):
                        nc.vector.reciprocal(linv, avp[hf][64:128, :])
                    nc.vector.tensor_mul(
                        oT_hat[pr][D * hf:D * (hf + 1),
                                   512 * qt:512 * (qt + 1)],
                        avp[hf][0:64, :], linv)
                step_filler(2)

            # ---------------- staged pipeline driver ----------------
            # Flat (rep, bc) pipeline: each bc's projections are scheduled
            # as fillers during the PREVIOUS bc's attention -- including
            # across the rep seam, so the steady-state per-rep time has no
            # projection-block bubble. Only the very first bc runs its
            # projections inline.
            def schedule_projections(stx):
                gens = []
                for jc in range(NEC):
                    for qt in range(NQT):
                        gens.append(gen_qkproj(stx, jc, qt))
                for tt in range(NTT):
                    gens.append(gen_vproj(stx, tt))
                pending.extend(gens)
                return gens

            st = make_state(0)
            for wname, wd in _late_weights:
                load_weight(wname, wd)
            for jc in range(NEC):
                for qt in range(NQT):
                    for _ in gen_qkproj(st, jc, qt):
                        pass
            for tt in range(NTT):
                for _ in gen_vproj(st, tt):
                    pass
            cur_gens = ()
            total_bcs = reps * BC
            for i in range(total_bcs):
                nxt = (make_state((i + 1) % BC)
                       if i + 1 < total_bcs else None)
                nxt_gens = schedule_projections(nxt) if nxt is not None else ()
                # this bc's own projections MUST be fully emitted before
                # its attention units read qT/kT/vsb
                force_gens(cur_gens)
                for qt in range(NQT):
                    for pr in range(NEC):
                        emit_attn_unit(st, pr, qt)
                    # this 512-query stripe of oT is complete for all
                    # heads -> its out^T halves become filler
                    for et in range(NEC):
                        pending.append(gen_outproj(st, et, qt))
                st, cur_gens = nxt, nxt_gens
            drain_all()

    nc.compile()
    return nc


_nc_lock = threading.Lock()
_nc_cache = None


def _get_nc():
    global _nc_cache
    with _nc_lock:
        if _nc_cache is None:
            _nc_cache = build_nc()
        return _nc_cache


def _make_in_maps(inputs):
    qT = np.ascontiguousarray(
        np.asarray(inputs["query"], np.float32).reshape(B * C, S, E)
        .transpose(0, 2, 1).astype(np.float16))
    kTf = np.ascontiguousarray(
        np.asarray(inputs["key"], np.float32).reshape(B * C, S, E)
        .transpose(0, 2, 1).astype(np.float16))
    vT = np.ascontiguousarray(
        np.asarray(inputs["value"], np.float32).reshape(B * C, S, E)
        .transpose(0, 2, 1).astype(np.float16))
    wqT = np.ascontiguousarray(np.asarray(inputs["Wq"], np.float32).T.astype(np.float16))
    wkT = np.ascontiguousarray(np.asarray(inputs["Wk"], np.float32).T.astype(np.float16))
    wvT = np.ascontiguousarray(np.asarray(inputs["Wv"], np.float32).T.astype(np.float16))
    woT = np.ascontiguousarray(np.asarray(inputs["Wo"], np.float32).T.astype(np.float16))
    bq = np.ascontiguousarray(np.asarray(inputs["bq"], np.float32))
    bk = np.ascontiguousarray(np.asarray(inputs["bk"], np.float32))
    bv = np.asarray(inputs["bv"], np.float32)
    bo = np.asarray(inputs["bo"], np.float32)
    # bv folds through the value path into an output-bias correction:
    #   (O + P @ (1 bv^T)) Wo^T + bo = O Wo^T + (bo + Wo bv)  [softmax rows sum to 1]
    wo = np.asarray(inputs["Wo"], np.float32)
    bo_eff = np.ascontiguousarray((bo + wo @ bv).astype(np.float32))
    in_maps = []
    for c in range(NCORES):
        sl = slice(BC * c, BC * (c + 1))
        in_maps.append({
            "q_in": np.ascontiguousarray(qT[sl]),
            "k_in": np.ascontiguousarray(kTf[sl]),
            "v_in": np.ascontiguousarray(vT[sl]),
            "wqT": wqT, "wkT": wkT, "wvT": wvT, "woT": woT,
            "bq": bq, "bk": bk, "bo_eff": bo_eff,
        })
    return in_maps


def kernel(**inputs):
    from concourse.bass_utils import run_bass_kernel_spmd

    nc = _get_nc()
    in_maps = _make_in_maps(inputs)
    res = run_bass_kernel_spmd(nc, in_maps, list(range(NCORES)))
    # device emits out^T [BC, E, S]; transpose back on the host
    outs = [res.results[i]["out"].transpose(0, 2, 1) for i in range(NCORES)]
    return np.concatenate(outs, axis=0).reshape(B, C, S, E).astype(np.float32)
